# revision 1
# baseline (speedup 1.0000x reference)
"""Trainium2 kernel for nn_M3oE: multi-domain MoE over 26 categorical embeddings.

Sharding: data-parallel over batch across 8 NeuronCores (2048 rows each),
embedding tables replicated in DRAM.

Gather strategy: the SWDGE small-descriptor path costs ~11ns/descriptor
serialized, so per-row (64B) indirect DMAs are the kernel wall.  Instead,
per (tile-group, field) we issue one batched `dma_gather` (Ant ucode) of
512 int16 block-indices, where a block = 4 consecutive table rows (256B,
the dma_gather minimum element).  Calls rotate over 4 SWDGE queues, which
overlaps their drain (~2.6x vs one queue).  The true row within each
gathered 4-row block is selected by a host-built {0,1} mask via a DVE
masked-reduction (y = sum_r S[...,r,:]*M_r), producing the same x layout
the rest of the pipeline used before.

Per-core pipeline (all matmuls in f32r = full-rate fp32):
  1. dma_gather of 512*26 4-row blocks per 512-sample tile group
  2. DVE masked-reduce -> x [128, (t,f,16)]
  3. PE transpose of x chunks -> xT [416(k-chunks), 512]
  4. 8 experts: h1T = relu(W1^T xT + b1) ; h2T = relu(W2^T h1T + b2),
     s_e = Wo . h2T accumulated into one [8, 512] PSUM tile
  5. domain-gated softmax; logits = (sum_e gsel_e * s_e)/denom + bo
"""

import ml_dtypes
import numpy as np

import concourse.bacc as bacc
import concourse.mybir as mybir
import concourse.tile as tile
from concourse.bass_utils import run_bass_kernel_spmd

F = 26
V = 100000
DK = 16
D = 4
E = 8
H1 = 128
H2 = 64
B = 16384
IN = F * DK  # 416
N_CORES = 8
PC = B // N_CORES  # 2048 rows per core
NT = PC // 128  # 16 batch-tiles of 128
NTG = 4  # tile groups
TGW = 512  # columns per tile group
TPG = NT // NTG  # 4 batch-tiles per group
KCH = [(0, 128), (128, 128), (256, 128), (384, 32)]  # k-chunks of IN=416
NQ = 4  # SWDGE queues for dma_gather rotation
BLK = 64  # f32 per gathered block (4 rows x 16)
NBF = V // 4  # 25000 blocks per field
NIX = TPG * 128  # 512 indices per gather call
IXW = NIX // 16  # 32 wrapped idx columns per call

F32 = mybir.dt.float32
F32R = mybir.dt.float32r
BF16 = mybir.dt.bfloat16
I32 = mybir.dt.int32
I16 = mybir.dt.int16

_cache = {}

# test-harness knobs (unused when the harness calls kernel() directly)
TRACE = False
LAST_RESULT = None


def _build(bo_val: float):
    nc = bacc.Bacc("TRN2", target_bir_lowering=False, debug=False,
                   num_devices=N_CORES, num_swdge_queues=NQ)

    emb4 = nc.dram_tensor("emb4", [F * NBF, BLK], F32, kind="ExternalInput")
    idx = nc.dram_tensor("idx16", [128, NTG * F * IXW], I16,
                         kind="ExternalInput")
    msk = nc.dram_tensor("msk", [128, NTG * TPG * F * 4], F32,
                         kind="ExternalInput")
    w1k = [nc.dram_tensor(f"w1k{i}", [w, E * H1], BF16, kind="ExternalInput")
           for i, (_, w) in enumerate(KCH)]
    w2c = nc.dram_tensor("w2c", [H1, E * H2], BF16, kind="ExternalInput")
    wo8 = nc.dram_tensor("wo8", [128, (E // 2) * 8], BF16,
                         kind="ExternalInput")
    wgk = [nc.dram_tensor(f"wgk{i}", [w, D * E], BF16, kind="ExternalInput")
           for i, (_, w) in enumerate(KCH)]
    sel8 = nc.dram_tensor("sel8", [D * E, 8], BF16, kind="ExternalInput")
    ones8 = nc.dram_tensor("ones8", [E, 1], BF16, kind="ExternalInput")
    ones32 = nc.dram_tensor("ones32", [D * E, 1], BF16, kind="ExternalInput")
    b1t = nc.dram_tensor("b1t", [H1, E], F32, kind="ExternalInput")
    b2t = nc.dram_tensor("b2t", [128, E // 2], F32, kind="ExternalInput")
    bgc = nc.dram_tensor("bgc", [D * E, 1], F32, kind="ExternalInput")
    oh = nc.dram_tensor("oh", [D * E, PC], F32, kind="ExternalInput")
    id128 = nc.dram_tensor("id128", [128, 128], BF16, kind="ExternalInput")
    out = nc.dram_tensor("out", [NTG, TGW], F32, kind="ExternalOutput")

    with tile.TileContext(nc) as tc:
        with (
            tc.tile_pool(name="const", bufs=1) as cpool,
            tc.tile_pool(name="stg", bufs=2) as stgpool,
            tc.tile_pool(name="mk", bufs=2) as mkpool,
            tc.tile_pool(name="yv", bufs=2) as ypool,
            tc.tile_pool(name="tmp", bufs=1) as tmppool,
            tc.tile_pool(name="xts", bufs=2 * len(KCH)) as xtspool,
            tc.tile_pool(name="h1s", bufs=3) as h1spool,
            tc.tile_pool(name="h2s", bufs=2) as h2spool,
            tc.tile_pool(name="gsb", bufs=2) as gsbpool,
            tc.tile_pool(name="fin", bufs=1) as finpool,
            tc.tile_pool(name="xtp", bufs=2, space="PSUM") as xtppool,
            tc.tile_pool(name="h1p", bufs=2, space="PSUM") as h1ppool,
            tc.tile_pool(name="h2p", bufs=1, space="PSUM") as h2ppool,
            tc.tile_pool(name="glp", bufs=1, space="PSUM") as glppool,
            tc.tile_pool(name="spp", bufs=1, space="PSUM") as sppool,
        ):
            # --- load constants ---
            def cload(dram, shape, dtype=None):
                t = cpool.tile(shape, dtype or dram.dtype, tag=dram.name)
                nc.sync.dma_start(out=t[:], in_=dram[:])
                return t

            # idx loaded in per-stage chunks so stage-0 gathers are not
            # gated on the full 852KB index DMA
            idx_sb = cpool.tile([128, NTG * F * IXW], I16, tag="idx16")
            for _s in range(NTG):
                _c0 = _s * F * IXW
                nc.sync.dma_start(out=idx_sb[:, _c0:_c0 + F * IXW],
                                  in_=idx[:, _c0:_c0 + F * IXW])
            w1_sb = [cload(w1k[i], [w, E * H1]) for i, (_, w) in enumerate(KCH)]
            w2_sb = cload(w2c, [H1, E * H2])
            wop_sb = cload(wo8, [128, (E // 2) * 8])
            wg_sb = [cload(wgk[i], [w, D * E]) for i, (_, w) in enumerate(KCH)]
            sel_sb = cload(sel8, [D * E, 8])
            on8_sb = cload(ones8, [E, 1])
            on32_sb = cload(ones32, [D * E, 1])
            b1_sb = cload(b1t, [H1, E])
            b2p_sb = cload(b2t, [128, E // 2])
            bg_sb = cload(bgc, [D * E, 1])
            oh_sb = cload(oh, [D * E, PC])
            id_sb = cload(id128, [128, 128])

            # --- emit ALL gathers first: they stream on the GpSimd queue,
            # paced only by S-buffer recycling (the masked-reduce frees S
            # early).  Compute is then emitted software-pipelined by one
            # group so group g+1's DVE reduce runs during group g's expert
            # matmuls instead of queueing behind g's final logits ops.
            qn = 0
            SM = []
            for tg in range(NTG):
                S = stgpool.tile([128, F * TPG * BLK], F32, tag="S")
                for f in range(F):
                    o0 = f * TPG * BLK
                    nc.gpsimd.dma_gather(
                        out_ap=S[:, o0:o0 + TPG * BLK].rearrange(
                            "p (g e) -> p g e", e=BLK),
                        in_ap=emb4[f * NBF:(f + 1) * NBF, :],
                        idxs_ap=idx_sb[:, (tg * F + f) * IXW:
                                       (tg * F + f + 1) * IXW],
                        num_idxs=NIX,
                        num_idxs_reg=NIX,
                        elem_size=BLK,
                        queue_num=qn % NQ,
                    )
                    qn += 1
                mk = mkpool.tile([128, TPG * F * 4], F32, tag="mk")
                nc.sync.dma_start(
                    out=mk[:],
                    in_=msk[:, tg * TPG * F * 4:(tg + 1) * TPG * F * 4])
                SM.append((S, mk))

            def emit_build_x(tg):
                S, mk = SM[tg]
                # --- select true row from each 4-row block:
                #     y[p,(t,f,d)] = sum_r S[p,(f,t,r,d)] * M[p,(t,f,r)] ---
                Sv = S[:].rearrange("p (f t r d) -> p t f r d",
                                    f=F, t=TPG, r=4, d=DK)
                Mv = mk[:].rearrange("p (t f r) -> p t f r", t=TPG, f=F)
                y = ypool.tile([128, TPG * F * DK], BF16, tag="y")
                t0 = tmppool.tile([128, TPG * F * DK], F32, tag="t0")
                t1 = tmppool.tile([128, TPG * F * DK], F32, tag="t1")

                def rsel(r):
                    return (Sv[:, :, :, r, :],
                            Mv[:, :, :, r:r + 1].broadcast_to(
                                [128, TPG, F, DK]))

                s0, m0 = rsel(0)
                nc.vector.tensor_tensor(out=t0[:], in0=s0, in1=m0,
                                        op=mybir.AluOpType.mult)
                s1, m1 = rsel(1)
                nc.vector.tensor_tensor(out=t1[:], in0=s1, in1=m1,
                                        op=mybir.AluOpType.mult)
                nc.vector.tensor_tensor(out=t0[:], in0=t0[:], in1=t1[:],
                                        op=mybir.AluOpType.add)
                s2, m2 = rsel(2)
                nc.vector.tensor_tensor(out=t1[:], in0=s2, in1=m2,
                                        op=mybir.AluOpType.mult)
                nc.vector.tensor_tensor(out=t0[:], in0=t0[:], in1=t1[:],
                                        op=mybir.AluOpType.add)
                s3, m3 = rsel(3)
                nc.vector.tensor_tensor(out=t1[:], in0=s3, in1=m3,
                                        op=mybir.AluOpType.mult)
                nc.vector.tensor_tensor(out=y[:], in0=t0[:], in1=t1[:],
                                        op=mybir.AluOpType.add)

                # --- transpose x -> xT per k-chunk (PE), evict to SBUF ---
                xts = []
                for kc, (koff, kw) in enumerate(KCH):
                    xtp = xtppool.tile([128, TGW], BF16, space="PSUM", tag="xtp")
                    for tl in range(TPG):
                        nc.tensor.transpose(
                            out=xtp[0:kw, tl * 128:(tl + 1) * 128],
                            in_=y[:, tl * IN + koff: tl * IN + koff + kw],
                            identity=id_sb[:],
                        )
                    xt = xtspool.tile([kw, TGW], BF16, tag=f"xts{kc}")
                    nc.vector.tensor_copy(xt[:], xtp[0:kw, :])
                    xts.append(xt)
                return xts

            def emit_C(tg, xts):
                # --- gating ---
                glp = glppool.tile([D * E, TGW], F32, space="PSUM", tag="glp")
                for kc in range(len(KCH)):
                    nc.tensor.matmul(glp[:], wg_sb[kc][:], xts[kc][:],
                                     start=(kc == 0), stop=(kc == len(KCH) - 1))
                expsb = gsbpool.tile([D * E, TGW], F32, tag="expsb")
                nc.scalar.activation(expsb[:], glp[:],
                                     mybir.ActivationFunctionType.Exp,
                                     bias=bg_sb[:, 0:1])
                masked = gsbpool.tile([D * E, TGW], BF16, tag="masked")
                nc.vector.tensor_tensor(
                    out=masked[:], in0=expsb[:],
                    in1=oh_sb[:, tg * TGW:(tg + 1) * TGW],
                    op=mybir.AluOpType.mult)
                gslp = glppool.tile([D * E, TGW], F32, space="PSUM", tag="glp")
                nc.tensor.matmul(gslp[0:8, :], sel_sb[:], masked[:],
                                 start=True, stop=True)
                gssb = gsbpool.tile([8, TGW], F32, tag="gssb")
                nc.scalar.activation(gssb[:], gslp[0:8, :],
                                     mybir.ActivationFunctionType.Copy)

                # --- experts ---
                sp = sppool.tile([E, 2 * TGW], F32, space="PSUM", tag="spp")
                for ep in range(E // 2):
                    h2p = h2ppool.tile([128, TGW], F32, space="PSUM",
                                       tag="h2p")
                    for sub in range(2):
                        e = 2 * ep + sub
                        h1p = h1ppool.tile([H1, TGW], F32, space="PSUM",
                                           tag="h1p")
                        for kc in range(len(KCH)):
                            nc.tensor.matmul(
                                h1p[:], w1_sb[kc][:, e * H1:(e + 1) * H1],
                                xts[kc][:],
                                start=(kc == 0), stop=(kc == len(KCH) - 1))
                        h1s = h1spool.tile([H1, TGW], BF16, tag="h1s")
                        nc.scalar.activation(
                            h1s[:], h1p[:],
                            mybir.ActivationFunctionType.Relu,
                            bias=b1_sb[:, e:e + 1])
                        nc.tensor.matmul(
                            h2p[sub * H2:(sub + 1) * H2, :],
                            w2_sb[:, e * H2:(e + 1) * H2],
                            h1s[:], start=True, stop=True,
                            skip_group_check=True)
                    h2s = h2spool.tile([128, TGW], BF16, tag="h2s")
                    nc.scalar.activation(h2s[:], h2p[:],
                                         mybir.ActivationFunctionType.Relu,
                                         bias=b2p_sb[:, ep:ep + 1])
                    nc.tensor.matmul(sp[:, 0:TGW],
                                     wop_sb[:, ep * 8:(ep + 1) * 8], h2s[:],
                                     start=(ep == 0), stop=(ep == E // 2 - 1),
                                     skip_group_check=True)
                return masked, gssb, sp

            def emit_D(tg, masked, gssb, sp):
                # --- final: logits = (sum_e gsel*s)/denom + bo ---
                msb = finpool.tile([E, TGW], BF16, tag="msb")
                nc.vector.tensor_tensor(out=msb[:], in0=sp[:, 0:TGW],
                                        in1=gssb[:],
                                        op=mybir.AluOpType.mult)
                updn = sppool.tile([E, 2 * TGW], F32, space="PSUM", tag="spp")
                nc.tensor.matmul(updn[0:1, 0:TGW], on8_sb[:], msb[:],
                                 start=True, stop=True)
                nc.tensor.matmul(updn[0:1, TGW:2 * TGW], on32_sb[:], masked[:],
                                 start=True, stop=True)
                rr = finpool.tile([1, TGW], F32, tag="rr")
                nc.vector.reciprocal(rr[:], updn[0:1, TGW:2 * TGW])
                lsb = finpool.tile([1, TGW], F32, tag="lsb")
                nc.vector.tensor_tensor(out=lsb[:], in0=updn[0:1, 0:TGW],
                                        in1=rr[:], op=mybir.AluOpType.mult)
                nc.vector.tensor_scalar_add(lsb[:], lsb[:], float(bo_val))
                nc.sync.dma_start(out=out[tg:tg + 1, :], in_=lsb[:])

            # Serial per-group emission measured fastest (222.7us): the
            # gather stream (emitted fully upfront) runs ahead on its own
            # queues; finer software-pipelining of compute phases was
            # tried and lost to SBUF/engine contention.
            for tg in range(NTG):
                xts = emit_build_x(tg)
                emit_D(tg, *emit_C(tg, xts))

    nc.compile()
    return nc


def kernel(**inputs):
    features = np.asarray(inputs["features"])
    domain = np.asarray(inputs["domain_indicator"])
    emb = np.asarray(inputs["emb"], dtype=np.float32)
    W1 = np.asarray(inputs["W1"], dtype=np.float32)
    b1 = np.asarray(inputs["b1"], dtype=np.float32)
    W2 = np.asarray(inputs["W2"], dtype=np.float32)
    b2 = np.asarray(inputs["b2"], dtype=np.float32)
    Wg = np.asarray(inputs["Wg"], dtype=np.float32)
    bg = np.asarray(inputs["bg"], dtype=np.float32)
    Wo = np.asarray(inputs["Wo"], dtype=np.float32)
    bo = np.asarray(inputs["bo"], dtype=np.float32)

    bo_val = float(bo.reshape(-1)[0])
    key = ("m3oe", bo_val)
    if key not in _cache:
        _cache[key] = _build(bo_val)
    nc = _cache[key]

    # ---- host-side prep (shared across cores) ----
    emb4_np = np.ascontiguousarray(emb.reshape(F * NBF, BLK))

    w1k = []
    wgk = []
    for koff, kw in KCH:
        w1k.append(np.ascontiguousarray(
            W1[:, koff:koff + kw, :].transpose(1, 0, 2).reshape(kw, E * H1)))
        wgk.append(np.ascontiguousarray(
            Wg[:, koff:koff + kw, :].transpose(1, 0, 2).reshape(kw, D * E)))
    w2c = np.ascontiguousarray(W2.transpose(1, 0, 2).reshape(H1, E * H2))
    # paired Wo: rows 0-63 = expert 2p (col 2p), rows 64-127 = expert
    # 2p+1 (col 2p+1), so one matmul reduces two experts' packed h2
    wo8 = np.zeros((128, (E // 2) * 8), dtype=np.float32)
    wov = Wo.reshape(H2)
    for ep in range(E // 2):
        wo8[0:H2, ep * 8 + 2 * ep] = wov
        wo8[H2:128, ep * 8 + 2 * ep + 1] = wov
    sel8 = np.zeros((D * E, 8), dtype=np.float32)
    for d in range(D):
        for e in range(E):
            sel8[d * 8 + e, e] = 1.0
    ones8 = np.ones((E, 1), dtype=np.float32)
    ones32 = np.ones((D * E, 1), dtype=np.float32)
    b1t = np.ascontiguousarray(b1.T)  # [H1, E]
    b2t = np.concatenate([b2[0::2].T, b2[1::2].T], axis=0)\
        .astype(np.float32)  # [128, E//2] stacked expert pairs
    bgc = bg.reshape(D * E, 1).astype(np.float32)
    id128 = np.eye(128, dtype=np.float32)

    bf = ml_dtypes.bfloat16
    shared = {
        "emb4": emb4_np,
        "w2c": w2c.astype(bf), "wo8": wo8.astype(bf),
        "sel8": sel8.astype(bf), "ones8": ones8.astype(bf),
        "ones32": ones32.astype(bf), "b1t": b1t, "b2t": b2t,
        "bgc": bgc, "id128": id128.astype(bf),
    }
    for i in range(len(KCH)):
        shared[f"w1k{i}"] = w1k[i].astype(bf)
        shared[f"wgk{i}"] = wgk[i].astype(bf)

    derep = np.repeat(np.arange(D), E)  # [32] domain of each (d,e) row
    lane = np.arange(128) % 16
    in_maps = []
    for c in range(N_CORES):
        sl = slice(c * PC, (c + 1) * PC)
        fc = features[sl].astype(np.int64)  # [PC, F]
        blk = (fc // 4).astype(np.int16)    # block index within field
        res = (fc % 4).astype(np.int64)     # row slot within block

        # idx16[p, (tg*F+f)*IXW + s] = blk[(tg*TPG+t)*128+p', f] where the
        # gather-order position i = t*128+p' is stored wrapped:
        # value at (p, s) is position s*16 + p%16.
        idx_core = np.zeros((128, NTG * F * IXW), dtype=np.int16)
        # blk reshaped per group: [NTG, TPG*128, F] -> positions i = t*128+p'
        blkg = blk.reshape(NTG, NIX, F)
        for tg in range(NTG):
            # w[s, lane, f] = blkg[tg, s*16+lane, f]
            w = blkg[tg].reshape(IXW, 16, F)
            # idx_core[p, ...] = w[s, p%16, f]
            block = w[:, lane, :]            # [IXW, 128, F]
            block = block.transpose(1, 2, 0)  # [128, F, IXW]
            idx_core[:, tg * F * IXW:(tg + 1) * F * IXW] = \
                block.reshape(128, F * IXW)

        # msk[p, (tg, t, f, r)]: 1.0 where sample (tg,t,p)'s field-f row
        # sits at slot r of its 4-row block
        resg = res.reshape(NTG, TPG, 128, F)  # [tg, t, p, f]
        onehot = (resg[:, :, :, :, None] ==
                  np.arange(4)).astype(np.float32)  # [tg, t, p, f, r]
        mk = np.ascontiguousarray(
            onehot.transpose(2, 0, 1, 3, 4).reshape(
                128, NTG * TPG * F * 4))

        dom = domain[sl].astype(np.int64)
        oh_core = (dom[None, :] == derep[:, None]).astype(np.float32)
        m = dict(shared)
        m["idx16"] = idx_core
        m["msk"] = mk
        m["oh"] = oh_core
        in_maps.append(m)

    global LAST_RESULT
    res_k = run_bass_kernel_spmd(nc, in_maps, core_ids=list(range(N_CORES)),
                                 trace=TRACE)
    LAST_RESULT = res_k
    outs = [res_k.results[c]["out"].reshape(PC) for c in range(N_CORES)]
    return np.concatenate(outs).astype(np.float32)



# revision 21
# speedup vs baseline: 1.0135x; 1.0135x over previous
"""Trainium2 kernel for nn_M3oE: multi-domain MoE over 26 categorical embeddings.

Sharding: data-parallel over batch across 8 NeuronCores (2048 rows each),
embedding tables replicated in DRAM.

v2 gather strategy: the per-call DMAGatherAnt descriptor-generation cost on
the Q7 (~1.2-2.5us/call) made the baseline's 104 calls the kernel wall
(137us GpSimd busy).  Now the table is stored bf16 as STRIDE-4 SLIDING
8-row windows (256B each, the dma_gather minimum element): window j holds
rows [4j..4j+7], so row r lives in window r//4 at slot r%4 and the per-field
window count (25000) fits int16.  One call per field gathers all 2048
samples -> 26 calls total.  The true row is selected from the 4 possible
slots by a host-built one-hot bf16 mask via a DVE mult/add tree, batched
over 8-field groups so each DVE op is [128, 2048] (dispatch-amortized).

Per-core pipeline (matmuls bf16):
  1. 26 dma_gather calls (4 SWDGE queues round-robin), field-group tiles
  2. per 8-field group: DVE 4-slot select -> y [128, (t,f,16)] bf16
  3. PE transpose of y chunks -> xT [416(k-chunks), 512] per sample group
  4. 8 experts: h1 = relu(W1^T xT + b1); h2 = relu(W2^T h1 + b2);
     s_e = Wo . h2 accumulated into one [8, 512] PSUM tile per group
  5. domain-gated softmax; logits = (num + bo*denom) * recip(denom),
     one batched [4, 512] DVE reciprocal for all sample groups
"""

import ml_dtypes
import numpy as np

import concourse.bacc as bacc
import concourse.mybir as mybir
import concourse.tile as tile
from concourse.bass_utils import run_bass_kernel_spmd

F = 26
V = 100000
DK = 16
D = 4
E = 8
H1 = 128
H2 = 64
B = 16384
IN = F * DK  # 416
N_CORES = 8
PC = B // N_CORES  # 2048 rows per core
NT = PC // 128  # 16 batch-tiles of 128
NTG = 4  # sample groups (matmul stage)
TGW = 512  # columns per sample group
KCH = [(0, 128), (128, 128), (256, 128), (384, 32)]  # k-chunks of IN=416
FGS = [(0, 8), (8, 8), (16, 8), (24, 2)]  # field groups (match KCH)
NQ = 4  # SWDGE queues for dma_gather rotation
WELEM = 128  # bf16 values per gathered window (8 rows x 16)
NW = V // 4  # 25000 stride-4 windows per field
VP = 4 * (NW - 1) + 8  # padded rows so the last window is in-bounds
IXW = PC // 16  # 128 wrapped idx columns per call

F32 = mybir.dt.float32
BF16 = mybir.dt.bfloat16
I16 = mybir.dt.int16

_cache = {}

# test-harness knobs (unused when the harness calls kernel() directly)
TRACE = False
LAST_RESULT = None


def _build():
    nc = bacc.Bacc("TRN2", target_bir_lowering=False, debug=False,
                   num_devices=N_CORES, num_swdge_queues=NQ)

    embS = nc.dram_tensor("embS", [F * NW, WELEM], BF16, kind="ExternalInput")
    idx = nc.dram_tensor("idx16", [128, F * IXW], I16, kind="ExternalInput")
    msk = nc.dram_tensor("mkb", [128, NT * F * 4], BF16, kind="ExternalInput")
    w1k = [nc.dram_tensor(f"w1k{i}", [w, E * H1], BF16, kind="ExternalInput")
           for i, (_, w) in enumerate(KCH)]
    w2c = nc.dram_tensor("w2c", [H1, E * H2], BF16, kind="ExternalInput")
    wo8 = nc.dram_tensor("wo8", [128, (E // 2) * 8], BF16,
                         kind="ExternalInput")
    wgk = [nc.dram_tensor(f"wgk{i}", [w, D * E], BF16, kind="ExternalInput")
           for i, (_, w) in enumerate(KCH)]
    sel8 = nc.dram_tensor("sel8", [D * E, 8], BF16, kind="ExternalInput")
    # routing matrices: group g's numerator -> nm_p row g, denominator ->
    # dn_p row g, bo*denom folded into the numerator rows
    on8s = nc.dram_tensor("on8s", [E, NTG * NTG], BF16, kind="ExternalInput")
    bo32s = nc.dram_tensor("bo32s", [D * E, NTG * NTG], BF16,
                           kind="ExternalInput")
    on32s = nc.dram_tensor("on32s", [D * E, NTG * NTG], BF16,
                           kind="ExternalInput")
    b1t = nc.dram_tensor("b1t", [H1, E], F32, kind="ExternalInput")
    b2t = nc.dram_tensor("b2t", [128, E // 2], F32, kind="ExternalInput")
    bgc = nc.dram_tensor("bgc", [D * E, 1], F32, kind="ExternalInput")
    oh = nc.dram_tensor("oh", [D * E, PC], F32, kind="ExternalInput")
    id128 = nc.dram_tensor("id128", [128, 128], BF16, kind="ExternalInput")
    out = nc.dram_tensor("out", [NTG, TGW], F32, kind="ExternalOutput")

    with tile.TileContext(nc) as tc:
        with (
            tc.tile_pool(name="const", bufs=1) as cpool,
            tc.tile_pool(name="stg", bufs=2) as stgpool,
            tc.tile_pool(name="yv", bufs=1) as ypool,
            tc.tile_pool(name="tmp", bufs=1) as tmppool,
            tc.tile_pool(name="xts", bufs=1) as xtspool,
            tc.tile_pool(name="h1s", bufs=3) as h1spool,
            tc.tile_pool(name="h2s", bufs=2) as h2spool,
            tc.tile_pool(name="gsb", bufs=2) as gsbpool,
            tc.tile_pool(name="fin", bufs=1) as finpool,
            tc.tile_pool(name="xtp", bufs=1, space="PSUM") as xtppool,
            tc.tile_pool(name="h1p", bufs=2, space="PSUM") as h1ppool,
            tc.tile_pool(name="h2p", bufs=1, space="PSUM") as h2ppool,
            tc.tile_pool(name="glp", bufs=1, space="PSUM") as glppool,
            tc.tile_pool(name="spp", bufs=1, space="PSUM") as sppool,
            tc.tile_pool(name="upd", bufs=1, space="PSUM") as updpool,
        ):
            # --- load constants ---
            def cload(dram, shape, dtype=None):
                t = cpool.tile(shape, dtype or dram.dtype, tag=dram.name)
                nc.sync.dma_start(out=t[:], in_=dram[:])
                return t

            # idx loaded in per-field-group chunks so the first gathers are
            # not gated on the full index DMA
            idx_sb = cpool.tile([128, F * IXW], I16, tag="idx16")
            for f0, nf in FGS:
                nc.sync.dma_start(
                    out=idx_sb[:, f0 * IXW:(f0 + nf) * IXW],
                    in_=idx[:, f0 * IXW:(f0 + nf) * IXW])
            mk_sb = cload(msk, [128, NT * F * 4])
            w1_sb = [cload(w1k[i], [w, E * H1]) for i, (_, w) in enumerate(KCH)]
            w2_sb = cload(w2c, [H1, E * H2])
            wop_sb = cload(wo8, [128, (E // 2) * 8])
            wg_sb = [cload(wgk[i], [w, D * E]) for i, (_, w) in enumerate(KCH)]
            sel_sb = cload(sel8, [D * E, 8])
            on8_sb = cload(on8s, [E, NTG * NTG])
            bo32_sb = cload(bo32s, [D * E, NTG * NTG])
            on32_sb = cload(on32s, [D * E, NTG * NTG])
            b1_sb = cload(b1t, [H1, E])
            b2p_sb = cload(b2t, [128, E // 2])
            bg_sb = cload(bgc, [D * E, 1])
            oh_sb = cload(oh, [D * E, PC])
            id_sb = cload(id128, [128, 128])

            # y holds the selected embeddings for ALL samples: [128,(t,f,d)]
            y = ypool.tile([128, NT * IN], BF16, tag="y")

            # --- emit ALL gathers first: they stream on the GpSimd queue,
            # paced by S-buffer recycling (select frees each group tile) ---
            SG = []
            qn = 0
            for f0, nf in FGS:
                S = stgpool.tile([128, nf * 16 * WELEM], BF16, tag="S")
                Sv = S[:].rearrange("p (f g e) -> p f g e",
                                    f=nf, g=16, e=WELEM)
                for fl in range(nf):
                    f = f0 + fl
                    nc.gpsimd.dma_gather(
                        out_ap=Sv[:, fl],
                        in_ap=embS[f * NW:(f + 1) * NW, :],
                        idxs_ap=idx_sb[:, f * IXW:(f + 1) * IXW],
                        num_idxs=PC,
                        num_idxs_reg=PC,
                        elem_size=WELEM,
                        # 2048 idxs -> 129 descriptors > the 64-desc packet
                        # ceiling; a single packet hangs the SDMA engine
                        single_packet=False,
                        queue_num=qn % NQ,
                    )
                    qn += 1
                SG.append(S)

            Mv = mk_sb[:].rearrange("p (t f s) -> p t f s", t=NT, f=F, s=4)
            yv = y[:].rearrange("p (t f d) -> p t f d", t=NT, f=F, d=DK)

            def emit_select(fgi):
                f0, nf = FGS[fgi]
                S = SG[fgi]
                # dims ordered (t, field, d) on both sides
                Sv = S[:].rearrange("p (f t e) -> p t f e",
                                    f=nf, t=16, e=WELEM)
                n = NT * nf * DK
                t0 = tmppool.tile([128, NT * 8 * DK], BF16, tag="t0")
                t1 = tmppool.tile([128, NT * 8 * DK], BF16, tag="t1")
                t0v = t0[:, 0:n].rearrange("p (t f d) -> p t f d",
                                           t=NT, f=nf, d=DK)
                t1v = t1[:, 0:n].rearrange("p (t f d) -> p t f d",
                                           t=NT, f=nf, d=DK)

                def rsel(s):
                    return (Sv[:, :, :, s * DK:(s + 1) * DK],
                            Mv[:, :, f0:f0 + nf, s:s + 1].broadcast_to(
                                [128, NT, nf, DK]))

                s0, m0 = rsel(0)
                nc.vector.tensor_tensor(out=t0v, in0=s0, in1=m0,
                                        op=mybir.AluOpType.mult)
                s1, m1 = rsel(1)
                nc.vector.tensor_tensor(out=t1v, in0=s1, in1=m1,
                                        op=mybir.AluOpType.mult)
                nc.vector.tensor_tensor(out=t0v, in0=t0v, in1=t1v,
                                        op=mybir.AluOpType.add)
                s2, m2 = rsel(2)
                nc.vector.tensor_tensor(out=t1v, in0=s2, in1=m2,
                                        op=mybir.AluOpType.mult)
                nc.vector.tensor_tensor(out=t0v, in0=t0v, in1=t1v,
                                        op=mybir.AluOpType.add)
                s3, m3 = rsel(3)
                nc.vector.tensor_tensor(out=t1v, in0=s3, in1=m3,
                                        op=mybir.AluOpType.mult)
                nc.vector.tensor_tensor(out=yv[:, :, f0:f0 + nf, :],
                                        in0=t0v, in1=t1v,
                                        op=mybir.AluOpType.add)

            # xts[g][kc] = xT chunk [kw, 512] for sample group g
            xts = [[None] * len(KCH) for _ in range(NTG)]

            def emit_transpose(kc):
                koff, kw = KCH[kc]
                for g in range(NTG):
                    xtp = xtppool.tile([128, TGW], BF16, space="PSUM",
                                       tag="xtp")
                    for tl in range(4):
                        t = 4 * g + tl
                        nc.tensor.transpose(
                            out=xtp[0:kw, tl * 128:(tl + 1) * 128],
                            in_=y[:, t * IN + koff: t * IN + koff + kw],
                            identity=id_sb[:],
                        )
                    xt = xtspool.tile([kw, TGW], BF16, tag=f"xts{kc}_{g}")
                    nc.scalar.activation(xt[:], xtp[0:kw, :],
                                         mybir.ActivationFunctionType.Copy)
                    xts[g][kc] = xt

            # field group fgi provides exactly k-chunk fgi (8 fields = 128
            # k-rows); select then transpose as each group of gathers lands
            for fgi in range(len(FGS)):
                emit_select(fgi)
                emit_transpose(fgi)

            def emit_C(g):
                xg = xts[g]
                # --- gating ---
                glp = glppool.tile([D * E, TGW], F32, space="PSUM", tag="glp")
                for kc in range(len(KCH)):
                    nc.tensor.matmul(glp[:], wg_sb[kc][:], xg[kc][:],
                                     start=(kc == 0), stop=(kc == len(KCH) - 1))
                expsb = gsbpool.tile([D * E, TGW], F32, tag="expsb")
                nc.scalar.activation(expsb[:], glp[:],
                                     mybir.ActivationFunctionType.Exp,
                                     bias=bg_sb[:, 0:1])
                masked = gsbpool.tile([D * E, TGW], BF16, tag="masked")
                nc.vector.tensor_tensor(
                    out=masked[:], in0=expsb[:],
                    in1=oh_sb[:, g * TGW:(g + 1) * TGW],
                    op=mybir.AluOpType.mult)
                gslp = glppool.tile([D * E, TGW], F32, space="PSUM", tag="glp")
                nc.tensor.matmul(gslp[0:8, :], sel_sb[:], masked[:],
                                 start=True, stop=True)
                gssb = gsbpool.tile([8, TGW], F32, tag="gssb")
                nc.scalar.activation(gssb[:], gslp[0:8, :],
                                     mybir.ActivationFunctionType.Copy)

                # --- experts ---
                sp = sppool.tile([E, TGW], F32, space="PSUM", tag="spp")
                for ep in range(E // 2):
                    h2p = h2ppool.tile([128, TGW], F32, space="PSUM",
                                       tag="h2p")
                    for sub in range(2):
                        e = 2 * ep + sub
                        h1p = h1ppool.tile([H1, TGW], F32, space="PSUM",
                                           tag="h1p")
                        for kc in range(len(KCH)):
                            nc.tensor.matmul(
                                h1p[:], w1_sb[kc][:, e * H1:(e + 1) * H1],
                                xg[kc][:],
                                start=(kc == 0), stop=(kc == len(KCH) - 1))
                        h1s = h1spool.tile([H1, TGW], BF16, tag="h1s")
                        nc.scalar.activation(
                            h1s[:], h1p[:],
                            mybir.ActivationFunctionType.Relu,
                            bias=b1_sb[:, e:e + 1])
                        nc.tensor.matmul(
                            h2p[sub * H2:(sub + 1) * H2, :],
                            w2_sb[:, e * H2:(e + 1) * H2],
                            h1s[:], start=True, stop=True,
                            skip_group_check=True)
                    h2s = h2spool.tile([128, TGW], BF16, tag="h2s")
                    nc.scalar.activation(h2s[:], h2p[:],
                                         mybir.ActivationFunctionType.Relu,
                                         bias=b2p_sb[:, ep:ep + 1])
                    nc.tensor.matmul(sp[:],
                                     wop_sb[:, ep * 8:(ep + 1) * 8], h2s[:],
                                     start=(ep == 0), stop=(ep == E // 2 - 1),
                                     skip_group_check=True)
                return masked, gssb, sp

            # shared [4, 512] PSUM tiles: nm_p row g = numerator + bo*denom
            # of group g, dn_p row g = denominator.  PE routing matrices
            # accumulate every group into them; one batched DVE reciprocal
            # at the end instead of 4x 1-partition reciprocals.
            nm_p = updpool.tile([NTG, TGW], F32, space="PSUM", tag="nm_p")
            dn_p = updpool.tile([NTG, TGW], F32, space="PSUM", tag="dn_p")

            def emit_D(g, masked, gssb, sp):
                msb = finpool.tile([E, TGW], BF16, tag="msb")
                nc.vector.tensor_tensor(out=msb[:], in0=sp[:],
                                        in1=gssb[:],
                                        op=mybir.AluOpType.mult)
                nc.tensor.matmul(nm_p[:], on8_sb[:, g * NTG:(g + 1) * NTG],
                                 msb[:], start=(g == 0), stop=False,
                                 skip_group_check=True)
                nc.tensor.matmul(nm_p[:], bo32_sb[:, g * NTG:(g + 1) * NTG],
                                 masked[:], start=False, stop=(g == NTG - 1),
                                 skip_group_check=True)
                nc.tensor.matmul(dn_p[:], on32_sb[:, g * NTG:(g + 1) * NTG],
                                 masked[:], start=(g == 0),
                                 stop=(g == NTG - 1),
                                 skip_group_check=True)

            for g in range(NTG):
                emit_D(g, *emit_C(g))
            rr = finpool.tile([NTG, TGW], F32, tag="rr")
            nc.vector.reciprocal(rr[:], dn_p[:])
            logits = finpool.tile([NTG, TGW], F32, tag="logits")
            nc.vector.tensor_tensor(out=logits[:], in0=nm_p[:],
                                    in1=rr[:], op=mybir.AluOpType.mult)
            nc.sync.dma_start(out=out[:], in_=logits[:])

    nc.compile()
    return nc


def kernel(**inputs):
    features = np.asarray(inputs["features"])
    domain = np.asarray(inputs["domain_indicator"])
    emb = np.asarray(inputs["emb"], dtype=np.float32)
    W1 = np.asarray(inputs["W1"], dtype=np.float32)
    b1 = np.asarray(inputs["b1"], dtype=np.float32)
    W2 = np.asarray(inputs["W2"], dtype=np.float32)
    b2 = np.asarray(inputs["b2"], dtype=np.float32)
    Wg = np.asarray(inputs["Wg"], dtype=np.float32)
    bg = np.asarray(inputs["bg"], dtype=np.float32)
    Wo = np.asarray(inputs["Wo"], dtype=np.float32)
    bo = np.asarray(inputs["bo"], dtype=np.float32)

    if "m3oe_v2" not in _cache:
        _cache["m3oe_v2"] = _build()
    nc = _cache["m3oe_v2"]

    bf = ml_dtypes.bfloat16
    bo_val = float(bo.reshape(-1)[0])

    # ---- host-side prep (shared across cores) ----
    # stride-4 sliding windows: window j of field f = rows [4j..4j+7] bf16
    embb = np.zeros((F, VP, DK), dtype=bf)
    embb[:, :V, :] = emb.astype(bf)
    s0, s1, s2 = embb.strides
    win = np.lib.stride_tricks.as_strided(
        embb, shape=(F, NW, 8, DK), strides=(s0, 4 * s1, s1, s2))
    embS = np.ascontiguousarray(win.reshape(F * NW, WELEM))

    w1k = []
    wgk = []
    for koff, kw in KCH:
        w1k.append(np.ascontiguousarray(
            W1[:, koff:koff + kw, :].transpose(1, 0, 2).reshape(kw, E * H1)))
        wgk.append(np.ascontiguousarray(
            Wg[:, koff:koff + kw, :].transpose(1, 0, 2).reshape(kw, D * E)))
    w2c = np.ascontiguousarray(W2.transpose(1, 0, 2).reshape(H1, E * H2))
    # paired Wo: rows 0-63 = expert 2p (col 2p), rows 64-127 = expert
    # 2p+1 (col 2p+1), so one matmul reduces two experts' packed h2
    wo8 = np.zeros((128, (E // 2) * 8), dtype=np.float32)
    wov = Wo.reshape(H2)
    for ep in range(E // 2):
        wo8[0:H2, ep * 8 + 2 * ep] = wov
        wo8[H2:128, ep * 8 + 2 * ep + 1] = wov
    sel8 = np.zeros((D * E, 8), dtype=np.float32)
    for d in range(D):
        for e in range(E):
            sel8[d * 8 + e, e] = 1.0
    on8s = np.zeros((E, NTG * NTG), dtype=np.float32)
    bo32s = np.zeros((D * E, NTG * NTG), dtype=np.float32)
    on32s = np.zeros((D * E, NTG * NTG), dtype=np.float32)
    for g in range(NTG):
        on8s[:, g * NTG + g] = 1.0         # numerator -> nm_p row g
        bo32s[:, g * NTG + g] = bo_val     # + bo * denom -> nm_p row g
        on32s[:, g * NTG + g] = 1.0        # denominator -> dn_p row g
    b1t = np.ascontiguousarray(b1.T)  # [H1, E]
    b2t = np.concatenate([b2[0::2].T, b2[1::2].T], axis=0)\
        .astype(np.float32)  # [128, E//2] stacked expert pairs
    bgc = bg.reshape(D * E, 1).astype(np.float32)
    id128 = np.eye(128, dtype=np.float32)

    shared = {
        "embS": embS,
        "w2c": w2c.astype(bf), "wo8": wo8.astype(bf),
        "sel8": sel8.astype(bf), "on8s": on8s.astype(bf),
        "bo32s": bo32s.astype(bf), "on32s": on32s.astype(bf),
        "b1t": b1t, "b2t": b2t, "bgc": bgc, "id128": id128.astype(bf),
    }
    for i in range(len(KCH)):
        shared[f"w1k{i}"] = w1k[i].astype(bf)
        shared[f"wgk{i}"] = wgk[i].astype(bf)

    derep = np.repeat(np.arange(D), E)  # [32] domain of each (d,e) row
    lane = np.arange(128) % 16
    in_maps = []
    for c in range(N_CORES):
        sl = slice(c * PC, (c + 1) * PC)
        fc = features[sl].astype(np.int64)  # [PC, F]
        wdx = (fc // 4).astype(np.int16)    # window index within field
        res = (fc % 4).astype(np.int64)     # slot within window

        # idx16[p, f*IXW + s] = wdx[s*16 + p%16, f] (wrapped in 16
        # partitions, replicated across Q7 cores)
        vw = wdx.reshape(IXW, 16, F)        # [s, lane, f]
        blk = vw[:, lane, :]                # [s, 128, f]
        idx_core = np.ascontiguousarray(
            blk.transpose(1, 2, 0).reshape(128, F * IXW))

        # mkb[p, (t, f, s)]: 1.0 where sample (t,p)'s field-f row sits at
        # slot s of its window
        resg = res.reshape(NT, 128, F)      # [t, p, f]
        onehot = (resg[:, :, :, None] == np.arange(4)).astype(bf)
        mk = np.ascontiguousarray(
            onehot.transpose(1, 0, 2, 3).reshape(128, NT * F * 4))

        dom = domain[sl].astype(np.int64)
        oh_core = (dom[None, :] == derep[:, None]).astype(np.float32)
        m = dict(shared)
        m["idx16"] = idx_core
        m["mkb"] = mk
        m["oh"] = oh_core
        in_maps.append(m)

    global LAST_RESULT
    res_k = run_bass_kernel_spmd(nc, in_maps, core_ids=list(range(N_CORES)),
                                 trace=TRACE)
    LAST_RESULT = res_k
    outs = [res_k.results[c]["out"].reshape(PC) for c in range(N_CORES)]
    return np.concatenate(outs).astype(np.float32)


# revision 25
# speedup vs baseline: 1.0347x; 1.0210x over previous
"""Trainium2 kernel for nn_M3oE: multi-domain MoE over 26 categorical embeddings.

Sharding: data-parallel over batch across 8 NeuronCores (2048 rows each),
embedding tables replicated in DRAM.

v2 gather strategy: the per-call DMAGatherAnt descriptor-generation cost on
the Q7 (~1.2-2.5us/call) made the baseline's 104 calls the kernel wall
(137us GpSimd busy).  Now the table is stored bf16 as STRIDE-4 SLIDING
8-row windows (256B each, the dma_gather minimum element): window j holds
rows [4j..4j+7], so row r lives in window r//4 at slot r%4 and the per-field
window count (25000) fits int16.  One call per field gathers all 2048
samples -> 26 calls total.  The true row is selected from the 4 possible
slots by a host-built one-hot bf16 mask via a DVE mult/add tree, batched
over 8-field groups so each DVE op is [128, 2048] (dispatch-amortized).

Per-core pipeline (matmuls bf16):
  1. 26 dma_gather calls (4 SWDGE queues round-robin), field-group tiles
  2. per 8-field group: DVE 4-slot select -> y [128, (t,f,16)] bf16
  3. PE transpose of y chunks -> xT [416(k-chunks), 512] per sample group
  4. 8 experts: h1 = relu(W1^T xT + b1); h2 = relu(W2^T h1 + b2);
     s_e = Wo . h2 accumulated into one [8, 512] PSUM tile per group
  5. domain-gated softmax; logits = (num + bo*denom) * recip(denom),
     one batched [4, 512] DVE reciprocal for all sample groups
"""

import ml_dtypes
import numpy as np

import concourse.bacc as bacc
import concourse.mybir as mybir
import concourse.tile as tile
from concourse.bass_utils import run_bass_kernel_spmd

F = 26
V = 100000
DK = 16
D = 4
E = 8
H1 = 128
H2 = 64
B = 16384
IN = F * DK  # 416
N_CORES = 8
PC = B // N_CORES  # 2048 rows per core
NT = PC // 128  # 16 batch-tiles of 128
NTG = 4  # sample groups (matmul stage)
TGW = 512  # columns per sample group
KCH = [(0, 128), (128, 128), (256, 128), (384, 32)]  # k-chunks of IN=416
# field groups of 4 (smaller S tiles); two groups feed one k-chunk
FGS = [(0, 4), (4, 4), (8, 4), (12, 4), (16, 4), (20, 4), (24, 2)]
FG_LAST_OF_KC = {1: 0, 3: 1, 5: 2, 6: 3}  # select fgi -> transpose kc
NQ = 4  # SWDGE queues for dma_gather rotation
WELEM = 128  # bf16 values per gathered window (8 rows x 16)
NW = V // 4  # 25000 stride-4 windows per field
VP = 4 * (NW - 1) + 8  # padded rows so the last window is in-bounds
IXW = PC // 16  # 128 wrapped idx columns per call

F32 = mybir.dt.float32
BF16 = mybir.dt.bfloat16
I16 = mybir.dt.int16

_cache = {}

# test-harness knobs (unused when the harness calls kernel() directly)
TRACE = False
LAST_RESULT = None


def _build():
    # 64KB descriptor carveout -> 4096-desc ring per queue: all 26 gathers'
    # descriptors (26 x ~258) fit without ring-full backpressure on the Q7
    nc = bacc.Bacc("TRN2", target_bir_lowering=False, debug=False,
                   num_devices=N_CORES, num_swdge_queues=NQ,
                   dynamic_dma_scratch_size=65536)

    embS = nc.dram_tensor("embS", [F * NW, WELEM], BF16, kind="ExternalInput")
    idx = nc.dram_tensor("idx16", [128, F * IXW], I16, kind="ExternalInput")
    msk = nc.dram_tensor("mkb", [128, NT * F * 4], BF16, kind="ExternalInput")
    w1k = [nc.dram_tensor(f"w1k{i}", [w, E * H1], BF16, kind="ExternalInput")
           for i, (_, w) in enumerate(KCH)]
    w2c = nc.dram_tensor("w2c", [H1, E * H2], BF16, kind="ExternalInput")
    wo8 = nc.dram_tensor("wo8", [128, (E // 2) * 8], BF16,
                         kind="ExternalInput")
    wgk = [nc.dram_tensor(f"wgk{i}", [w, D * E], BF16, kind="ExternalInput")
           for i, (_, w) in enumerate(KCH)]
    sel8 = nc.dram_tensor("sel8", [D * E, 8], BF16, kind="ExternalInput")
    # routing matrices: group g's numerator -> nm_p row g, denominator ->
    # dn_p row g, bo*denom folded into the numerator rows
    on8s = nc.dram_tensor("on8s", [E, NTG * NTG], BF16, kind="ExternalInput")
    bo32s = nc.dram_tensor("bo32s", [D * E, NTG * NTG], BF16,
                           kind="ExternalInput")
    on32s = nc.dram_tensor("on32s", [D * E, NTG * NTG], BF16,
                           kind="ExternalInput")
    b1t = nc.dram_tensor("b1t", [H1, E], F32, kind="ExternalInput")
    b2t = nc.dram_tensor("b2t", [128, E // 2], F32, kind="ExternalInput")
    bgc = nc.dram_tensor("bgc", [D * E, 1], F32, kind="ExternalInput")
    oh = nc.dram_tensor("oh", [D * E, PC], F32, kind="ExternalInput")
    id128 = nc.dram_tensor("id128", [128, 128], BF16, kind="ExternalInput")
    out = nc.dram_tensor("out", [NTG, TGW], F32, kind="ExternalOutput")

    with tile.TileContext(nc) as tc:
        with (
            tc.tile_pool(name="const", bufs=1) as cpool,
            tc.tile_pool(name="stg", bufs=2) as stgpool,
            tc.tile_pool(name="yv", bufs=1) as ypool,
            tc.tile_pool(name="tmp", bufs=1) as tmppool,
            tc.tile_pool(name="xts", bufs=1) as xtspool,
            tc.tile_pool(name="h1s", bufs=3) as h1spool,
            tc.tile_pool(name="h2s", bufs=2) as h2spool,
            tc.tile_pool(name="gsb", bufs=2) as gsbpool,
            tc.tile_pool(name="fin", bufs=1) as finpool,
            tc.tile_pool(name="xtp", bufs=1, space="PSUM") as xtppool,
            tc.tile_pool(name="h1p", bufs=2, space="PSUM") as h1ppool,
            tc.tile_pool(name="h2p", bufs=1, space="PSUM") as h2ppool,
            tc.tile_pool(name="glp", bufs=1, space="PSUM") as glppool,
            tc.tile_pool(name="spp", bufs=1, space="PSUM") as sppool,
            tc.tile_pool(name="upd", bufs=1, space="PSUM") as updpool,
        ):
            # --- load constants ---
            def cload(dram, shape, dtype=None):
                t = cpool.tile(shape, dtype or dram.dtype, tag=dram.name)
                nc.sync.dma_start(out=t[:], in_=dram[:])
                return t

            # idx loaded in per-field-group chunks so the first gathers are
            # not gated on the full index DMA
            idx_sb = cpool.tile([128, F * IXW], I16, tag="idx16")
            for f0, nf in FGS:
                nc.sync.dma_start(
                    out=idx_sb[:, f0 * IXW:(f0 + nf) * IXW],
                    in_=idx[:, f0 * IXW:(f0 + nf) * IXW])
            mk_sb = cload(msk, [128, NT * F * 4])
            w1_sb = [cload(w1k[i], [w, E * H1]) for i, (_, w) in enumerate(KCH)]
            w2_sb = cload(w2c, [H1, E * H2])
            wop_sb = cload(wo8, [128, (E // 2) * 8])
            wg_sb = [cload(wgk[i], [w, D * E]) for i, (_, w) in enumerate(KCH)]
            sel_sb = cload(sel8, [D * E, 8])
            on8_sb = cload(on8s, [E, NTG * NTG])
            bo32_sb = cload(bo32s, [D * E, NTG * NTG])
            on32_sb = cload(on32s, [D * E, NTG * NTG])
            b1_sb = cload(b1t, [H1, E])
            b2p_sb = cload(b2t, [128, E // 2])
            bg_sb = cload(bgc, [D * E, 1])
            oh_sb = cload(oh, [D * E, PC])
            id_sb = cload(id128, [128, 128])

            # y holds the selected embeddings for ALL samples: [128,(t,f,d)]
            y = ypool.tile([128, NT * IN], BF16, tag="y")

            # --- emit ALL gathers first: they stream on the GpSimd queue,
            # paced by S-buffer recycling (select frees each group tile) ---
            SG = []
            qn = 0
            for f0, nf in FGS:
                S = stgpool.tile([128, nf * 16 * WELEM], BF16, tag="S")
                Sv = S[:].rearrange("p (f g e) -> p f g e",
                                    f=nf, g=16, e=WELEM)
                for fl in range(nf):
                    f = f0 + fl
                    nc.gpsimd.dma_gather(
                        out_ap=Sv[:, fl],
                        in_ap=embS[f * NW:(f + 1) * NW, :],
                        idxs_ap=idx_sb[:, f * IXW:(f + 1) * IXW],
                        num_idxs=PC,
                        num_idxs_reg=PC,
                        elem_size=WELEM,
                        # 2048 idxs -> 129 descriptors > the 64-desc packet
                        # ceiling; a single packet hangs the SDMA engine
                        single_packet=False,
                        queue_num=qn % NQ,
                    )
                    qn += 1
                SG.append(S)

            Mv = mk_sb[:].rearrange("p (t f s) -> p t f s", t=NT, f=F, s=4)
            yv = y[:].rearrange("p (t f d) -> p t f d", t=NT, f=F, d=DK)

            def emit_select(fgi):
                f0, nf = FGS[fgi]
                S = SG[fgi]
                # dims ordered (t, field, d) on both sides
                Sv = S[:].rearrange("p (f t e) -> p t f e",
                                    f=nf, t=16, e=WELEM)
                n = NT * nf * DK
                t0 = tmppool.tile([128, NT * 4 * DK], BF16, tag="t0")
                t1 = tmppool.tile([128, NT * 4 * DK], BF16, tag="t1")
                t0v = t0[:, 0:n].rearrange("p (t f d) -> p t f d",
                                           t=NT, f=nf, d=DK)
                t1v = t1[:, 0:n].rearrange("p (t f d) -> p t f d",
                                           t=NT, f=nf, d=DK)

                def rsel(s):
                    return (Sv[:, :, :, s * DK:(s + 1) * DK],
                            Mv[:, :, f0:f0 + nf, s:s + 1].broadcast_to(
                                [128, NT, nf, DK]))

                s0, m0 = rsel(0)
                nc.vector.tensor_tensor(out=t0v, in0=s0, in1=m0,
                                        op=mybir.AluOpType.mult)
                s1, m1 = rsel(1)
                nc.vector.tensor_tensor(out=t1v, in0=s1, in1=m1,
                                        op=mybir.AluOpType.mult)
                nc.vector.tensor_tensor(out=t0v, in0=t0v, in1=t1v,
                                        op=mybir.AluOpType.add)
                s2, m2 = rsel(2)
                nc.vector.tensor_tensor(out=t1v, in0=s2, in1=m2,
                                        op=mybir.AluOpType.mult)
                nc.vector.tensor_tensor(out=t0v, in0=t0v, in1=t1v,
                                        op=mybir.AluOpType.add)
                s3, m3 = rsel(3)
                nc.vector.tensor_tensor(out=t1v, in0=s3, in1=m3,
                                        op=mybir.AluOpType.mult)
                nc.vector.tensor_tensor(out=yv[:, :, f0:f0 + nf, :],
                                        in0=t0v, in1=t1v,
                                        op=mybir.AluOpType.add)

            # xts[g][kc] = xT chunk [kw, 512] for sample group g
            xts = [[None] * len(KCH) for _ in range(NTG)]

            def emit_transpose(kc):
                koff, kw = KCH[kc]
                for g in range(NTG):
                    xtp = xtppool.tile([128, TGW], BF16, space="PSUM",
                                       tag="xtp")
                    for tl in range(4):
                        t = 4 * g + tl
                        nc.tensor.transpose(
                            out=xtp[0:kw, tl * 128:(tl + 1) * 128],
                            in_=y[:, t * IN + koff: t * IN + koff + kw],
                            identity=id_sb[:],
                        )
                    xt = xtspool.tile([kw, TGW], BF16, tag=f"xts{kc}_{g}")
                    nc.scalar.activation(xt[:], xtp[0:kw, :],
                                         mybir.ActivationFunctionType.Copy)
                    xts[g][kc] = xt

            # two field groups (8 fields = 128 k-rows) feed one k-chunk;
            # select then transpose as the gathers land
            for fgi in range(len(FGS)):
                emit_select(fgi)
                if fgi in FG_LAST_OF_KC:
                    emit_transpose(FG_LAST_OF_KC[fgi])

            def emit_C(g):
                xg = xts[g]
                # --- gating ---
                glp = glppool.tile([D * E, TGW], F32, space="PSUM", tag="glp")
                for kc in range(len(KCH)):
                    nc.tensor.matmul(glp[:], wg_sb[kc][:], xg[kc][:],
                                     start=(kc == 0), stop=(kc == len(KCH) - 1))
                expsb = gsbpool.tile([D * E, TGW], F32, tag="expsb")
                nc.scalar.activation(expsb[:], glp[:],
                                     mybir.ActivationFunctionType.Exp,
                                     bias=bg_sb[:, 0:1])
                masked = gsbpool.tile([D * E, TGW], BF16, tag="masked")
                nc.vector.tensor_tensor(
                    out=masked[:], in0=expsb[:],
                    in1=oh_sb[:, g * TGW:(g + 1) * TGW],
                    op=mybir.AluOpType.mult)
                gslp = glppool.tile([D * E, TGW], F32, space="PSUM", tag="glp")
                nc.tensor.matmul(gslp[0:8, :], sel_sb[:], masked[:],
                                 start=True, stop=True)
                gssb = gsbpool.tile([8, TGW], F32, tag="gssb")
                nc.scalar.activation(gssb[:], gslp[0:8, :],
                                     mybir.ActivationFunctionType.Copy)

                # --- experts ---
                sp = sppool.tile([E, TGW], F32, space="PSUM", tag="spp")
                for ep in range(E // 2):
                    h2p = h2ppool.tile([128, TGW], F32, space="PSUM",
                                       tag="h2p")
                    for sub in range(2):
                        e = 2 * ep + sub
                        h1p = h1ppool.tile([H1, TGW], F32, space="PSUM",
                                           tag="h1p")
                        for kc in range(len(KCH)):
                            nc.tensor.matmul(
                                h1p[:], w1_sb[kc][:, e * H1:(e + 1) * H1],
                                xg[kc][:],
                                start=(kc == 0), stop=(kc == len(KCH) - 1))
                        h1s = h1spool.tile([H1, TGW], BF16, tag="h1s")
                        nc.scalar.activation(
                            h1s[:], h1p[:],
                            mybir.ActivationFunctionType.Relu,
                            bias=b1_sb[:, e:e + 1])
                        nc.tensor.matmul(
                            h2p[sub * H2:(sub + 1) * H2, :],
                            w2_sb[:, e * H2:(e + 1) * H2],
                            h1s[:], start=True, stop=True,
                            skip_group_check=True)
                    h2s = h2spool.tile([128, TGW], BF16, tag="h2s")
                    nc.scalar.activation(h2s[:], h2p[:],
                                         mybir.ActivationFunctionType.Relu,
                                         bias=b2p_sb[:, ep:ep + 1])
                    nc.tensor.matmul(sp[:],
                                     wop_sb[:, ep * 8:(ep + 1) * 8], h2s[:],
                                     start=(ep == 0), stop=(ep == E // 2 - 1),
                                     skip_group_check=True)
                return masked, gssb, sp

            # shared [4, 512] PSUM tiles: nm_p row g = numerator + bo*denom
            # of group g, dn_p row g = denominator.  PE routing matrices
            # accumulate every group into them; one batched DVE reciprocal
            # at the end instead of 4x 1-partition reciprocals.
            nm_p = updpool.tile([NTG, TGW], F32, space="PSUM", tag="nm_p")
            dn_p = updpool.tile([NTG, TGW], F32, space="PSUM", tag="dn_p")

            def emit_D(g, masked, gssb, sp):
                msb = finpool.tile([E, TGW], BF16, tag="msb")
                nc.vector.tensor_tensor(out=msb[:], in0=sp[:],
                                        in1=gssb[:],
                                        op=mybir.AluOpType.mult)
                nc.tensor.matmul(nm_p[:], on8_sb[:, g * NTG:(g + 1) * NTG],
                                 msb[:], start=(g == 0), stop=False,
                                 skip_group_check=True)
                nc.tensor.matmul(nm_p[:], bo32_sb[:, g * NTG:(g + 1) * NTG],
                                 masked[:], start=False, stop=(g == NTG - 1),
                                 skip_group_check=True)
                nc.tensor.matmul(dn_p[:], on32_sb[:, g * NTG:(g + 1) * NTG],
                                 masked[:], start=(g == 0),
                                 stop=(g == NTG - 1),
                                 skip_group_check=True)

            for g in range(NTG):
                emit_D(g, *emit_C(g))
            rr = finpool.tile([NTG, TGW], F32, tag="rr")
            nc.vector.reciprocal(rr[:], dn_p[:])
            logits = finpool.tile([NTG, TGW], F32, tag="logits")
            nc.vector.tensor_tensor(out=logits[:], in0=nm_p[:],
                                    in1=rr[:], op=mybir.AluOpType.mult)
            nc.sync.dma_start(out=out[:], in_=logits[:])

    nc.compile()
    return nc


def kernel(**inputs):
    features = np.asarray(inputs["features"])
    domain = np.asarray(inputs["domain_indicator"])
    emb = np.asarray(inputs["emb"], dtype=np.float32)
    W1 = np.asarray(inputs["W1"], dtype=np.float32)
    b1 = np.asarray(inputs["b1"], dtype=np.float32)
    W2 = np.asarray(inputs["W2"], dtype=np.float32)
    b2 = np.asarray(inputs["b2"], dtype=np.float32)
    Wg = np.asarray(inputs["Wg"], dtype=np.float32)
    bg = np.asarray(inputs["bg"], dtype=np.float32)
    Wo = np.asarray(inputs["Wo"], dtype=np.float32)
    bo = np.asarray(inputs["bo"], dtype=np.float32)

    if "m3oe_v2" not in _cache:
        _cache["m3oe_v2"] = _build()
    nc = _cache["m3oe_v2"]

    bf = ml_dtypes.bfloat16
    bo_val = float(bo.reshape(-1)[0])

    # ---- host-side prep (shared across cores) ----
    # stride-4 sliding windows: window j of field f = rows [4j..4j+7] bf16
    embb = np.zeros((F, VP, DK), dtype=bf)
    embb[:, :V, :] = emb.astype(bf)
    s0, s1, s2 = embb.strides
    win = np.lib.stride_tricks.as_strided(
        embb, shape=(F, NW, 8, DK), strides=(s0, 4 * s1, s1, s2))
    embS = np.ascontiguousarray(win.reshape(F * NW, WELEM))

    w1k = []
    wgk = []
    for koff, kw in KCH:
        w1k.append(np.ascontiguousarray(
            W1[:, koff:koff + kw, :].transpose(1, 0, 2).reshape(kw, E * H1)))
        wgk.append(np.ascontiguousarray(
            Wg[:, koff:koff + kw, :].transpose(1, 0, 2).reshape(kw, D * E)))
    w2c = np.ascontiguousarray(W2.transpose(1, 0, 2).reshape(H1, E * H2))
    # paired Wo: rows 0-63 = expert 2p (col 2p), rows 64-127 = expert
    # 2p+1 (col 2p+1), so one matmul reduces two experts' packed h2
    wo8 = np.zeros((128, (E // 2) * 8), dtype=np.float32)
    wov = Wo.reshape(H2)
    for ep in range(E // 2):
        wo8[0:H2, ep * 8 + 2 * ep] = wov
        wo8[H2:128, ep * 8 + 2 * ep + 1] = wov
    sel8 = np.zeros((D * E, 8), dtype=np.float32)
    for d in range(D):
        for e in range(E):
            sel8[d * 8 + e, e] = 1.0
    on8s = np.zeros((E, NTG * NTG), dtype=np.float32)
    bo32s = np.zeros((D * E, NTG * NTG), dtype=np.float32)
    on32s = np.zeros((D * E, NTG * NTG), dtype=np.float32)
    for g in range(NTG):
        on8s[:, g * NTG + g] = 1.0         # numerator -> nm_p row g
        bo32s[:, g * NTG + g] = bo_val     # + bo * denom -> nm_p row g
        on32s[:, g * NTG + g] = 1.0        # denominator -> dn_p row g
    b1t = np.ascontiguousarray(b1.T)  # [H1, E]
    b2t = np.concatenate([b2[0::2].T, b2[1::2].T], axis=0)\
        .astype(np.float32)  # [128, E//2] stacked expert pairs
    bgc = bg.reshape(D * E, 1).astype(np.float32)
    id128 = np.eye(128, dtype=np.float32)

    shared = {
        "embS": embS,
        "w2c": w2c.astype(bf), "wo8": wo8.astype(bf),
        "sel8": sel8.astype(bf), "on8s": on8s.astype(bf),
        "bo32s": bo32s.astype(bf), "on32s": on32s.astype(bf),
        "b1t": b1t, "b2t": b2t, "bgc": bgc, "id128": id128.astype(bf),
    }
    for i in range(len(KCH)):
        shared[f"w1k{i}"] = w1k[i].astype(bf)
        shared[f"wgk{i}"] = wgk[i].astype(bf)

    derep = np.repeat(np.arange(D), E)  # [32] domain of each (d,e) row
    lane = np.arange(128) % 16
    in_maps = []
    for c in range(N_CORES):
        sl = slice(c * PC, (c + 1) * PC)
        fc = features[sl].astype(np.int64)  # [PC, F]
        wdx = (fc // 4).astype(np.int16)    # window index within field
        res = (fc % 4).astype(np.int64)     # slot within window

        # idx16[p, f*IXW + s] = wdx[s*16 + p%16, f] (wrapped in 16
        # partitions, replicated across Q7 cores)
        vw = wdx.reshape(IXW, 16, F)        # [s, lane, f]
        blk = vw[:, lane, :]                # [s, 128, f]
        idx_core = np.ascontiguousarray(
            blk.transpose(1, 2, 0).reshape(128, F * IXW))

        # mkb[p, (t, f, s)]: 1.0 where sample (t,p)'s field-f row sits at
        # slot s of its window
        resg = res.reshape(NT, 128, F)      # [t, p, f]
        onehot = (resg[:, :, :, None] == np.arange(4)).astype(bf)
        mk = np.ascontiguousarray(
            onehot.transpose(1, 0, 2, 3).reshape(128, NT * F * 4))

        dom = domain[sl].astype(np.int64)
        oh_core = (dom[None, :] == derep[:, None]).astype(np.float32)
        m = dict(shared)
        m["idx16"] = idx_core
        m["mkb"] = mk
        m["oh"] = oh_core
        in_maps.append(m)

    global LAST_RESULT
    res_k = run_bass_kernel_spmd(nc, in_maps, core_ids=list(range(N_CORES)),
                                 trace=TRACE)
    LAST_RESULT = res_k
    outs = [res_k.results[c]["out"].reshape(PC) for c in range(N_CORES)]
    return np.concatenate(outs).astype(np.float32)


# revision 34
# speedup vs baseline: 1.1739x; 1.1345x over previous
"""Trainium2 kernel for nn_M3oE: multi-domain MoE over 26 categorical embeddings.

Sharding: data-parallel over batch across 8 NeuronCores (2048 rows each),
embedding tables replicated in DRAM.

Gather: the table is stored bf16 as STRIDE-4 SLIDING 8-row windows (256B
each, the dma_gather minimum element): window j holds rows [4j..4j+7], so
row r lives in window r//4 at slot r%4 and the per-field window count
(25000) fits int16.  Measured on HW the gather is HBM-random-read bound
(~2.2ns/lookup regardless of call batching; const-idx is 3x SLOWER due to
bank serialization, sorted idx no better), so the gather phase ~120us/core
is a floor.  The kernel therefore pipelines SAMPLE QUARTERS: gathers are
ordered (quarter, field) in 104 calls of 512 idxs, and each quarter's
select/transpose/expert stage runs under the next quarters' gather stream,
leaving only the last quarter's compute (~25us) as tail.

Per-core pipeline (matmuls bf16):
  1. 104 dma_gather calls (4 SWDGE queues round-robin), (quarter,
     field-half) staging tiles
  2. per (quarter, 13-field half): DVE 4-slot one-hot select -> y slice
  3. PE transpose of y chunks -> xT [416(k-chunks), 512] per quarter
  4. 8 experts: h1 = relu(W1^T xT + b1); h2 = relu(W2^T h1 + b2);
     s_e = Wo . h2 accumulated into one [8, 512] PSUM tile per quarter
  5. domain-gated softmax; logits = (num + bo*denom) * recip(denom),
     one batched [4, 512] DVE reciprocal for all quarters
"""

import ml_dtypes
import numpy as np

import concourse.bacc as bacc
import concourse.mybir as mybir
import concourse.tile as tile
from concourse.bass_utils import run_bass_kernel_spmd

F = 26
V = 100000
DK = 16
D = 4
E = 8
H1 = 128
H2 = 64
B = 16384
IN = F * DK  # 416
N_CORES = 8
PC = B // N_CORES  # 2048 rows per core
NT = PC // 128  # 16 batch-tiles of 128
NTG = 4  # sample groups (matmul stage)
TGW = 512  # columns per sample group
KCH = [(0, 128), (128, 128), (256, 128), (384, 32)]  # k-chunks of IN=416
NTQ = NT // NTG  # 4 batch-tiles (512 samples) per quarter
FH = [(0, 13), (13, 13)]  # field halves for the batched select
NQ = 4  # SWDGE queues for dma_gather rotation
WELEM = 128  # bf16 values per gathered window (8 rows x 16)
NW = V // 4  # 25000 stride-4 windows per field
VP = 4 * (NW - 1) + 8  # padded rows so the last window is in-bounds
IXQ = TGW // 16  # 32 wrapped idx columns per (quarter, field) call

F32 = mybir.dt.float32
BF16 = mybir.dt.bfloat16
I16 = mybir.dt.int16

_cache = {}

# test-harness knobs (unused when the harness calls kernel() directly)
TRACE = False
LAST_RESULT = None


def _build():
    # 64KB descriptor carveout -> 4096-desc ring per queue: all 26 gathers'
    # descriptors (26 x ~258) fit without ring-full backpressure on the Q7
    nc = bacc.Bacc("TRN2", target_bir_lowering=False, debug=False,
                   num_devices=N_CORES, num_swdge_queues=NQ,
                   dynamic_dma_scratch_size=65536)

    embS = nc.dram_tensor("embS", [F * NW, WELEM], BF16, kind="ExternalInput")
    idx = nc.dram_tensor("idx16", [128, NTG * F * IXQ], I16,
                         kind="ExternalInput")
    msk = nc.dram_tensor("mkb", [128, NT * F * 4], BF16, kind="ExternalInput")
    w1k = [nc.dram_tensor(f"w1k{i}", [w, E * H1], BF16, kind="ExternalInput")
           for i, (_, w) in enumerate(KCH)]
    w2c = nc.dram_tensor("w2c", [H1, E * H2], BF16, kind="ExternalInput")
    wo8 = nc.dram_tensor("wo8", [128, (E // 2) * 8], BF16,
                         kind="ExternalInput")
    wgk = [nc.dram_tensor(f"wgk{i}", [w, D * E], BF16, kind="ExternalInput")
           for i, (_, w) in enumerate(KCH)]
    sel8 = nc.dram_tensor("sel8", [D * E, 8], BF16, kind="ExternalInput")
    # routing matrices: group g's numerator -> nm_p row g, denominator ->
    # dn_p row g, bo*denom folded into the numerator rows
    on8s = nc.dram_tensor("on8s", [E, NTG * NTG], BF16, kind="ExternalInput")
    bo32s = nc.dram_tensor("bo32s", [D * E, NTG * NTG], BF16,
                           kind="ExternalInput")
    on32s = nc.dram_tensor("on32s", [D * E, NTG * NTG], BF16,
                           kind="ExternalInput")
    b1t = nc.dram_tensor("b1t", [H1, E], F32, kind="ExternalInput")
    b2t = nc.dram_tensor("b2t", [128, E // 2], F32, kind="ExternalInput")
    bgc = nc.dram_tensor("bgc", [D * E, 1], F32, kind="ExternalInput")
    oh = nc.dram_tensor("oh", [D * E, PC], F32, kind="ExternalInput")
    id128 = nc.dram_tensor("id128", [128, 128], BF16, kind="ExternalInput")
    out = nc.dram_tensor("out", [NTG, TGW], F32, kind="ExternalOutput")

    with tile.TileContext(nc) as tc:
        with (
            tc.tile_pool(name="const", bufs=1) as cpool,
            tc.tile_pool(name="stg", bufs=3) as stgpool,
            tc.tile_pool(name="yv", bufs=1) as ypool,
            tc.tile_pool(name="tmp", bufs=1) as tmppool,
            tc.tile_pool(name="xts", bufs=1) as xtspool,
            tc.tile_pool(name="h1s", bufs=3) as h1spool,
            tc.tile_pool(name="h2s", bufs=2) as h2spool,
            tc.tile_pool(name="gsb", bufs=2) as gsbpool,
            tc.tile_pool(name="fin", bufs=1) as finpool,
            tc.tile_pool(name="xtp", bufs=1, space="PSUM") as xtppool,
            tc.tile_pool(name="h1p", bufs=2, space="PSUM") as h1ppool,
            tc.tile_pool(name="h2p", bufs=1, space="PSUM") as h2ppool,
            tc.tile_pool(name="glp", bufs=1, space="PSUM") as glppool,
            tc.tile_pool(name="spp", bufs=1, space="PSUM") as sppool,
            tc.tile_pool(name="upd", bufs=1, space="PSUM") as updpool,
        ):
            # --- load constants ---
            def cload(dram, shape, dtype=None):
                t = cpool.tile(shape, dtype or dram.dtype, tag=dram.name)
                nc.sync.dma_start(out=t[:], in_=dram[:])
                return t

            # idx loaded in per-quarter chunks so the first gathers are
            # not gated on the full index DMA
            idx_sb = cpool.tile([128, NTG * F * IXQ], I16, tag="idx16")
            for q in range(NTG):
                nc.sync.dma_start(
                    out=idx_sb[:, q * F * IXQ:(q + 1) * F * IXQ],
                    in_=idx[:, q * F * IXQ:(q + 1) * F * IXQ])
            mk_sb = cload(msk, [128, NT * F * 4])
            w1_sb = [cload(w1k[i], [w, E * H1]) for i, (_, w) in enumerate(KCH)]
            w2_sb = cload(w2c, [H1, E * H2])
            wop_sb = cload(wo8, [128, (E // 2) * 8])
            wg_sb = [cload(wgk[i], [w, D * E]) for i, (_, w) in enumerate(KCH)]
            sel_sb = cload(sel8, [D * E, 8])
            on8_sb = cload(on8s, [E, NTG * NTG])
            bo32_sb = cload(bo32s, [D * E, NTG * NTG])
            on32_sb = cload(on32s, [D * E, NTG * NTG])
            b1_sb = cload(b1t, [H1, E])
            b2p_sb = cload(b2t, [128, E // 2])
            bg_sb = cload(bgc, [D * E, 1])
            oh_sb = cload(oh, [D * E, PC])
            id_sb = cload(id128, [128, 128])

            # y holds the selected embeddings for ALL samples: [128,(t,f,d)]
            y = ypool.tile([128, NT * IN], BF16, tag="y")

            # --- emit ALL gathers first, ordered (quarter, field): each
            # quarter's expert stage then overlaps the next quarters'
            # gather stream.  Paced by S-buffer recycling. ---
            SQ = [[None, None] for _ in range(NTG)]
            qn = 0
            for q in range(NTG):
                for h, (f0, nf) in enumerate(FH):
                    S = stgpool.tile([128, nf * NTQ * WELEM], BF16, tag="S")
                    Sv = S[:].rearrange("p (f g e) -> p f g e",
                                        f=nf, g=NTQ, e=WELEM)
                    for fl in range(nf):
                        f = f0 + fl
                        nc.gpsimd.dma_gather(
                            out_ap=Sv[:, fl],
                            in_ap=embS[f * NW:(f + 1) * NW, :],
                            idxs_ap=idx_sb[:, (q * F + f) * IXQ:
                                           (q * F + f + 1) * IXQ],
                            num_idxs=TGW,
                            num_idxs_reg=TGW,
                            elem_size=WELEM,
                            single_packet=False,
                            queue_num=qn % NQ,
                        )
                        qn += 1
                    SQ[q][h] = S

            Mv = mk_sb[:].rearrange("p (t f s) -> p t f s", t=NT, f=F, s=4)
            yv = y[:].rearrange("p (t f d) -> p t f d", t=NT, f=F, d=DK)

            def emit_select(q, h):
                f0, nf = FH[h]
                S = SQ[q][h]
                # dims ordered (t, field, d) on both sides
                Sv = S[:].rearrange("p (f t e) -> p t f e",
                                    f=nf, t=NTQ, e=WELEM)
                n = NTQ * nf * DK
                t0 = tmppool.tile([128, NTQ * 13 * DK], BF16, tag="t0")
                t1 = tmppool.tile([128, NTQ * 13 * DK], BF16, tag="t1")
                t0v = t0[:, 0:n].rearrange("p (t f d) -> p t f d",
                                           t=NTQ, f=nf, d=DK)
                t1v = t1[:, 0:n].rearrange("p (t f d) -> p t f d",
                                           t=NTQ, f=nf, d=DK)

                def rsel(s):
                    return (Sv[:, :, :, s * DK:(s + 1) * DK],
                            Mv[:, NTQ * q:NTQ * (q + 1), f0:f0 + nf,
                               s:s + 1].broadcast_to(
                                [128, NTQ, nf, DK]))

                s0, m0 = rsel(0)
                nc.vector.tensor_tensor(out=t0v, in0=s0, in1=m0,
                                        op=mybir.AluOpType.mult)
                s1, m1 = rsel(1)
                nc.vector.tensor_tensor(out=t1v, in0=s1, in1=m1,
                                        op=mybir.AluOpType.mult)
                nc.vector.tensor_tensor(out=t0v, in0=t0v, in1=t1v,
                                        op=mybir.AluOpType.add)
                s2, m2 = rsel(2)
                nc.vector.tensor_tensor(out=t1v, in0=s2, in1=m2,
                                        op=mybir.AluOpType.mult)
                nc.vector.tensor_tensor(out=t0v, in0=t0v, in1=t1v,
                                        op=mybir.AluOpType.add)
                s3, m3 = rsel(3)
                nc.vector.tensor_tensor(out=t1v, in0=s3, in1=m3,
                                        op=mybir.AluOpType.mult)
                nc.vector.tensor_tensor(
                    out=yv[:, NTQ * q:NTQ * (q + 1), f0:f0 + nf, :],
                    in0=t0v, in1=t1v, op=mybir.AluOpType.add)

            # xts[g][kc] = xT chunk [kw, 512] for sample group g
            xts = [[None] * len(KCH) for _ in range(NTG)]

            def emit_transpose_q(q):
                for kc, (koff, kw) in enumerate(KCH):
                    xtp = xtppool.tile([128, TGW], BF16, space="PSUM",
                                       tag="xtp")
                    for tl in range(NTQ):
                        t = NTQ * q + tl
                        nc.tensor.transpose(
                            out=xtp[0:kw, tl * 128:(tl + 1) * 128],
                            in_=y[:, t * IN + koff: t * IN + koff + kw],
                            identity=id_sb[:],
                        )
                    xt = xtspool.tile([kw, TGW], BF16, tag=f"xts{kc}_{q}")
                    nc.scalar.activation(xt[:], xtp[0:kw, :],
                                         mybir.ActivationFunctionType.Copy)
                    xts[q][kc] = xt

            def emit_C(g):
                xg = xts[g]
                # --- gating ---
                glp = glppool.tile([D * E, TGW], F32, space="PSUM", tag="glp")
                for kc in range(len(KCH)):
                    nc.tensor.matmul(glp[:], wg_sb[kc][:], xg[kc][:],
                                     start=(kc == 0), stop=(kc == len(KCH) - 1))
                expsb = gsbpool.tile([D * E, TGW], F32, tag="expsb")
                nc.scalar.activation(expsb[:], glp[:],
                                     mybir.ActivationFunctionType.Exp,
                                     bias=bg_sb[:, 0:1])
                masked = gsbpool.tile([D * E, TGW], BF16, tag="masked")
                nc.vector.tensor_tensor(
                    out=masked[:], in0=expsb[:],
                    in1=oh_sb[:, g * TGW:(g + 1) * TGW],
                    op=mybir.AluOpType.mult)
                gslp = glppool.tile([D * E, TGW], F32, space="PSUM", tag="glp")
                nc.tensor.matmul(gslp[0:8, :], sel_sb[:], masked[:],
                                 start=True, stop=True)
                gssb = gsbpool.tile([8, TGW], F32, tag="gssb")
                nc.scalar.activation(gssb[:], gslp[0:8, :],
                                     mybir.ActivationFunctionType.Copy)

                # --- experts ---
                sp = sppool.tile([E, TGW], F32, space="PSUM", tag="spp")
                for ep in range(E // 2):
                    h2p = h2ppool.tile([128, TGW], F32, space="PSUM",
                                       tag="h2p")
                    for sub in range(2):
                        e = 2 * ep + sub
                        h1p = h1ppool.tile([H1, TGW], F32, space="PSUM",
                                           tag="h1p")
                        for kc in range(len(KCH)):
                            nc.tensor.matmul(
                                h1p[:], w1_sb[kc][:, e * H1:(e + 1) * H1],
                                xg[kc][:],
                                start=(kc == 0), stop=(kc == len(KCH) - 1))
                        h1s = h1spool.tile([H1, TGW], BF16, tag="h1s")
                        nc.scalar.activation(
                            h1s[:], h1p[:],
                            mybir.ActivationFunctionType.Relu,
                            bias=b1_sb[:, e:e + 1])
                        nc.tensor.matmul(
                            h2p[sub * H2:(sub + 1) * H2, :],
                            w2_sb[:, e * H2:(e + 1) * H2],
                            h1s[:], start=True, stop=True,
                            skip_group_check=True)
                    h2s = h2spool.tile([128, TGW], BF16, tag="h2s")
                    nc.scalar.activation(h2s[:], h2p[:],
                                         mybir.ActivationFunctionType.Relu,
                                         bias=b2p_sb[:, ep:ep + 1])
                    nc.tensor.matmul(sp[:],
                                     wop_sb[:, ep * 8:(ep + 1) * 8], h2s[:],
                                     start=(ep == 0), stop=(ep == E // 2 - 1),
                                     skip_group_check=True)
                return masked, gssb, sp

            # shared [4, 512] PSUM tiles: nm_p row g = numerator + bo*denom
            # of group g, dn_p row g = denominator.  PE routing matrices
            # accumulate every group into them; one batched DVE reciprocal
            # at the end instead of 4x 1-partition reciprocals.
            nm_p = updpool.tile([NTG, TGW], F32, space="PSUM", tag="nm_p")
            dn_p = updpool.tile([NTG, TGW], F32, space="PSUM", tag="dn_p")

            def emit_D(g, masked, gssb, sp):
                msb = finpool.tile([E, TGW], BF16, tag="msb")
                nc.vector.tensor_tensor(out=msb[:], in0=sp[:],
                                        in1=gssb[:],
                                        op=mybir.AluOpType.mult)
                nc.tensor.matmul(nm_p[:], on8_sb[:, g * NTG:(g + 1) * NTG],
                                 msb[:], start=(g == 0), stop=False,
                                 skip_group_check=True)
                nc.tensor.matmul(nm_p[:], bo32_sb[:, g * NTG:(g + 1) * NTG],
                                 masked[:], start=False, stop=(g == NTG - 1),
                                 skip_group_check=True)
                nc.tensor.matmul(dn_p[:], on32_sb[:, g * NTG:(g + 1) * NTG],
                                 masked[:], start=(g == 0),
                                 stop=(g == NTG - 1),
                                 skip_group_check=True)

            for q in range(NTG):
                emit_select(q, 0)
                emit_select(q, 1)
                emit_transpose_q(q)
                emit_D(q, *emit_C(q))
            rr = finpool.tile([NTG, TGW], F32, tag="rr")
            nc.vector.reciprocal(rr[:], dn_p[:])
            logits = finpool.tile([NTG, TGW], F32, tag="logits")
            nc.vector.tensor_tensor(out=logits[:], in0=nm_p[:],
                                    in1=rr[:], op=mybir.AluOpType.mult)
            nc.sync.dma_start(out=out[:], in_=logits[:])

    nc.compile()
    return nc


def kernel(**inputs):
    features = np.asarray(inputs["features"])
    domain = np.asarray(inputs["domain_indicator"])
    emb = np.asarray(inputs["emb"], dtype=np.float32)
    W1 = np.asarray(inputs["W1"], dtype=np.float32)
    b1 = np.asarray(inputs["b1"], dtype=np.float32)
    W2 = np.asarray(inputs["W2"], dtype=np.float32)
    b2 = np.asarray(inputs["b2"], dtype=np.float32)
    Wg = np.asarray(inputs["Wg"], dtype=np.float32)
    bg = np.asarray(inputs["bg"], dtype=np.float32)
    Wo = np.asarray(inputs["Wo"], dtype=np.float32)
    bo = np.asarray(inputs["bo"], dtype=np.float32)

    if "m3oe_v2" not in _cache:
        _cache["m3oe_v2"] = _build()
    nc = _cache["m3oe_v2"]

    bf = ml_dtypes.bfloat16
    bo_val = float(bo.reshape(-1)[0])

    # ---- host-side prep (shared across cores) ----
    # stride-4 sliding windows: window j of field f = rows [4j..4j+7] bf16
    embb = np.zeros((F, VP, DK), dtype=bf)
    embb[:, :V, :] = emb.astype(bf)
    s0, s1, s2 = embb.strides
    win = np.lib.stride_tricks.as_strided(
        embb, shape=(F, NW, 8, DK), strides=(s0, 4 * s1, s1, s2))
    embS = np.ascontiguousarray(win.reshape(F * NW, WELEM))

    w1k = []
    wgk = []
    for koff, kw in KCH:
        w1k.append(np.ascontiguousarray(
            W1[:, koff:koff + kw, :].transpose(1, 0, 2).reshape(kw, E * H1)))
        wgk.append(np.ascontiguousarray(
            Wg[:, koff:koff + kw, :].transpose(1, 0, 2).reshape(kw, D * E)))
    w2c = np.ascontiguousarray(W2.transpose(1, 0, 2).reshape(H1, E * H2))
    # paired Wo: rows 0-63 = expert 2p (col 2p), rows 64-127 = expert
    # 2p+1 (col 2p+1), so one matmul reduces two experts' packed h2
    wo8 = np.zeros((128, (E // 2) * 8), dtype=np.float32)
    wov = Wo.reshape(H2)
    for ep in range(E // 2):
        wo8[0:H2, ep * 8 + 2 * ep] = wov
        wo8[H2:128, ep * 8 + 2 * ep + 1] = wov
    sel8 = np.zeros((D * E, 8), dtype=np.float32)
    for d in range(D):
        for e in range(E):
            sel8[d * 8 + e, e] = 1.0
    on8s = np.zeros((E, NTG * NTG), dtype=np.float32)
    bo32s = np.zeros((D * E, NTG * NTG), dtype=np.float32)
    on32s = np.zeros((D * E, NTG * NTG), dtype=np.float32)
    for g in range(NTG):
        on8s[:, g * NTG + g] = 1.0         # numerator -> nm_p row g
        bo32s[:, g * NTG + g] = bo_val     # + bo * denom -> nm_p row g
        on32s[:, g * NTG + g] = 1.0        # denominator -> dn_p row g
    b1t = np.ascontiguousarray(b1.T)  # [H1, E]
    b2t = np.concatenate([b2[0::2].T, b2[1::2].T], axis=0)\
        .astype(np.float32)  # [128, E//2] stacked expert pairs
    bgc = bg.reshape(D * E, 1).astype(np.float32)
    id128 = np.eye(128, dtype=np.float32)

    shared = {
        "embS": embS,
        "w2c": w2c.astype(bf), "wo8": wo8.astype(bf),
        "sel8": sel8.astype(bf), "on8s": on8s.astype(bf),
        "bo32s": bo32s.astype(bf), "on32s": on32s.astype(bf),
        "b1t": b1t, "b2t": b2t, "bgc": bgc, "id128": id128.astype(bf),
    }
    for i in range(len(KCH)):
        shared[f"w1k{i}"] = w1k[i].astype(bf)
        shared[f"wgk{i}"] = wgk[i].astype(bf)

    derep = np.repeat(np.arange(D), E)  # [32] domain of each (d,e) row
    lane = np.arange(128) % 16
    in_maps = []
    for c in range(N_CORES):
        sl = slice(c * PC, (c + 1) * PC)
        fc = features[sl].astype(np.int64)  # [PC, F]
        wdx = (fc // 4).astype(np.int16)    # window index within field
        res = (fc % 4).astype(np.int64)     # slot within window

        # idx16[p, (q*F+f)*IXQ + s] = wdx[q*512 + s*16 + p%16, f]
        # (wrapped in 16 partitions, replicated across Q7 cores)
        vw = wdx.reshape(NTG, IXQ, 16, F)   # [q, s, lane, f]
        blk = vw[:, :, lane, :]             # [q, s, 128, f]
        idx_core = np.ascontiguousarray(
            blk.transpose(2, 0, 3, 1).reshape(128, NTG * F * IXQ))

        # mkb[p, (t, f, s)]: 1.0 where sample (t,p)'s field-f row sits at
        # slot s of its window
        resg = res.reshape(NT, 128, F)      # [t, p, f]
        onehot = (resg[:, :, :, None] == np.arange(4)).astype(bf)
        mk = np.ascontiguousarray(
            onehot.transpose(1, 0, 2, 3).reshape(128, NT * F * 4))

        dom = domain[sl].astype(np.int64)
        oh_core = (dom[None, :] == derep[:, None]).astype(np.float32)
        m = dict(shared)
        m["idx16"] = idx_core
        m["mkb"] = mk
        m["oh"] = oh_core
        in_maps.append(m)

    global LAST_RESULT
    res_k = run_bass_kernel_spmd(nc, in_maps, core_ids=list(range(N_CORES)),
                                 trace=TRACE)
    LAST_RESULT = res_k
    outs = [res_k.results[c]["out"].reshape(PC) for c in range(N_CORES)]
    return np.concatenate(outs).astype(np.float32)
